# revision 1
# baseline (speedup 1.0000x reference)
"""Trainium2 Bass kernel for nn_ACOPFEnforcer (per-node-type MLP, no message passing).

Math per node type t (sizes SB=4000, PQ=200000, PV=80000, NB=116000):
    inp = concat(x_t, c_t)                      # [N, 11]
    z_l = inp @ W[l,t] + b[l,t]                 # l = 0..2, [N, 128]
    h_l = ELU(z_l)
    P_l = h_l[:, :64].sum(-1); Q_l = h_l[:, 64:].sum(-1)
    out[l*N+n] = ELU(P_l[n]*W2[0] + Q_l[n]*W2[1] + b2)   # [128]
Output = concat over types of the [3*N_t, 128] blocks.

Strategy: pure data parallelism over nodes across 8 NeuronCores.  On-chip
everything runs "orientation B" (channels on partitions, nodes on the free
dim).  ELU is computed exactly via the identity  ELU(z) = min(exp(z)-1, relu(z)),
one ScalarE Exp + one drain (ScalarE Relu or VectorE tensor_scalar, mixed for
load balance) + one fused VectorE scalar_tensor_tensor.  P/Q sums are
matmuls against a 0/1 mask; four subgroups of PQ rows are packed into one
PSUM bank at partition offsets {0,32,64,96} (col-tiled matmuls) so a single
copy drains them.  Stage-2 matmul uses float32r (full-rate fp32) with
tile_position row groups.  Biases ride for free in the per-partition
activation bias / tensor_scalar scalar operand.  Output is written bf16 in
[channel, node] layout and transposed/cast on the host.
"""

import os
import numpy as np
import ml_dtypes

import concourse.bass as bass
import concourse.tile as tile
from concourse import bacc, mybir
from concourse.bass_utils import run_bass_kernel_spmd

BF16 = mybir.dt.bfloat16
F32 = mybir.dt.float32
F32R = mybir.dt.float32r
AF = mybir.ActivationFunctionType
OP = mybir.AluOpType

NODE_TYPES = ["SB", "PQ", "PV", "NB"]
SIZES = {"SB": 4000, "PQ": 200000, "PV": 80000, "NB": 116000}
NUM_LAYERS = 3
N_CORES = 8
TILE_N = 1024          # nodes per stage-1 tile (one z psum buffer, 2 banks)
SUB = 512              # nodes per matmul / stage-2 subgroup
# padded per-type sizes: multiple of 8*TILE_N so each core gets whole tiles
PAD = {t: -(-SIZES[t] // (N_CORES * TILE_N)) * (N_CORES * TILE_N) for t in NODE_TYPES}
PPC = {t: PAD[t] // N_CORES for t in NODE_TYPES}        # padded nodes/core
VPC = {t: SIZES[t] // N_CORES for t in NODE_TYPES}      # valid nodes/core
OUT_COLS = sum(NUM_LAYERS * PPC[t] for t in NODE_TYPES)  # per-core out cols

# fraction of drain2 ops routed to the vector engine (tunable load balance)
R1_DVE = float(os.environ.get("K_R1", "0.65"))   # stage 1
R2_DVE = float(os.environ.get("K_R2", "0.65"))   # stage 2
POOL_C = float(os.environ.get("K_P", "0.0"))     # pool share of combines

_CACHE = {}


def _build_nc():
    nc = bacc.Bacc("TRN2", target_bir_lowering=False, debug=False,
                   enable_asserts=False, num_devices=N_CORES)

    inp_ap = {t: nc.dram_tensor(f"inp_{t}", [11, PPC[t]], BF16,
                                kind="ExternalInput").ap() for t in NODE_TYPES}
    wpack_ap = nc.dram_tensor("wpack", [11, NUM_LAYERS * 4 * 128], BF16,
                              kind="ExternalInput").ap()
    bfc_ap = nc.dram_tensor("bfc", [128, NUM_LAYERS * 4], F32,
                            kind="ExternalInput").ap()
    mask_ap = nc.dram_tensor("mask", [128, 2], BF16, kind="ExternalInput").ap()
    w2c_ap = nc.dram_tensor("w2c", [98, 128], F32R, kind="ExternalInput").ap()
    bfcp1_ap = nc.dram_tensor("bfcp1", [128, NUM_LAYERS * 4], F32,
                              kind="ExternalInput").ap()
    b2adj_ap = nc.dram_tensor("b2adj", [128, 1], F32, kind="ExternalInput").ap()
    b2adjp1_ap = nc.dram_tensor("b2adjp1", [128, 1], F32,
                                kind="ExternalInput").ap()
    out_ap = nc.dram_tensor("out", [128, OUT_COLS], BF16,
                            kind="ExternalOutput").ap()

    with tile.TileContext(nc) as tc:
        _emit(tc, inp_ap, wpack_ap, bfc_ap, bfcp1_ap, mask_ap, w2c_ap, b2adj_ap, b2adjp1_ap, out_ap)
    nc.compile()
    return nc


def _emit(tc, inp_ap, wpack_ap, bfc_ap, bfcp1_ap, mask_ap, w2c_ap, b2adj_ap, b2adjp1_ap, out_ap):
    nc = tc.nc
    from contextlib import ExitStack
    ctx = ExitStack()
    with ctx:
        consts = ctx.enter_context(tc.tile_pool(name="consts", bufs=1))
        p_inp = ctx.enter_context(tc.tile_pool(name="inp", bufs=3))
        p_e1 = ctx.enter_context(tc.tile_pool(name="e1", bufs=3))
        p_t1 = ctx.enter_context(tc.tile_pool(name="t1", bufs=3))
        p_r1 = ctx.enter_context(tc.tile_pool(name="r1", bufs=3))
        p_pq1 = ctx.enter_context(tc.tile_pool(name="pq1", bufs=4))
        p_e2 = ctx.enter_context(tc.tile_pool(name="e2", bufs=3))
        p_r2 = ctx.enter_context(tc.tile_pool(name="r2", bufs=3))
        p_out = ctx.enter_context(tc.tile_pool(name="osb", bufs=3))
        ps_z = ctx.enter_context(tc.tile_pool(name="zps", bufs=2, space="PSUM"))
        ps_pq = ctx.enter_context(tc.tile_pool(name="pqps", bufs=2, space="PSUM"))
        ps_y = ctx.enter_context(tc.tile_pool(name="yps", bufs=1, space="PSUM"))

        # constants
        wpack = consts.tile([11, NUM_LAYERS * 4 * 128], BF16, tag="wpack", name="wpack")
        nc.sync.dma_start(wpack[:], wpack_ap[:])
        bfc = consts.tile([128, NUM_LAYERS * 4], F32, tag="bfc", name="bfc")
        nc.sync.dma_start(bfc[:], bfc_ap[:])
        mask = consts.tile([128, 2], BF16, tag="mask", name="mask")
        nc.sync.dma_start(mask[:], mask_ap[:])
        w2c = consts.tile([98, 128], F32R, tag="w2c", name="w2c")
        nc.sync.dma_start(w2c[:], w2c_ap[:])
        bfcp1 = consts.tile([128, NUM_LAYERS * 4], F32, tag="bfcp1", name="bfcp1")
        nc.sync.dma_start(bfcp1[:], bfcp1_ap[:])
        b2adj = consts.tile([128, 1], F32, tag="b2adj", name="b2adj")
        nc.sync.dma_start(b2adj[:], b2adj_ap[:])
        b2adjp1 = consts.tile([128, 1], F32, tag="b2adjp1", name="b2adjp1")
        nc.sync.dma_start(b2adjp1[:], b2adjp1_ap[:])

        # pq packing state: 4 subgroups -> one psum bank at partitions {0,32,64,96}
        pack = {"ps": None, "n": 0, "consumers": [], "k": 0}
        pair_i = [0]  # stage-2 pair counter (drain-engine mixing)
        gl_i = [0]    # global (g,l) counter
        osb_pending = {}

        def osb_done_one(osb_key):
            rec = osb_pending[osb_key]
            rec[2] -= 1
            if rec[2] == 0:
                nc.sync.dma_start(out_ap[:, rec[1]:rec[1] + rec[3]], rec[0][:, 0:rec[3]])
                del osb_pending[osb_key]

        def flush_pack():
            if pack["n"] == 0:
                return
            pq_sb = p_pq1.tile([98, SUB], F32R, tag="pq1", name="pq1")
            if pack["k"] % 2 == 0:
                nc.vector.tensor_copy(pq_sb[0:98, :], pack["ps"][0:98, :])
            else:
                nc.scalar.copy(pq_sb[0:98, :], pack["ps"][0:98, :])
            pack["k"] += 1
            cons = pack["consumers"]
            for a in range(0, len(cons), 2):
                if a + 1 < len(cons):
                    j0, f0 = cons[a]
                    j1, f1 = cons[a + 1]
                    assert f0 is f1 and j1 == j0 + 1
                    f0(pq_sb, j0, True)
                else:
                    j0, f0 = cons[a]
                    f0(pq_sb, j0, False)
            pack["ps"] = None
            pack["n"] = 0
            pack["consumers"] = []

        def add_to_pack(t1, h_off, stage2_fn):
            if pack["ps"] is None:
                pack["ps"] = ps_pq.tile([128, SUB], F32, tag="pqps", name="pqps")
            j = pack["n"]
            # P/Q sums over T1=ELU+1 (offset folded into stage-2 bias)
            nc.tensor.matmul(pack["ps"][32 * j:32 * j + 2, :],
                             lhsT=mask[:, :], rhs=t1[:, h_off:h_off + SUB],
                             start=True, stop=True,
                             tile_position=(0, 32 * j))
            pack["consumers"].append((j, stage2_fn))
            pack["n"] += 1
            if pack["n"] == 4:
                flush_pack()

        OBATCH = 4   # (g,l) tiles batched into one output DMA

        for _rep in range(int(os.environ.get("K_REPEAT", "1"))):
         for ti, t in enumerate(NODE_TYPES):
            n_tiles = PPC[t] // TILE_N
            tbase = sum(NUM_LAYERS * PPC[u] for u in NODE_TYPES[:ti])
            for l in range(NUM_LAYERS):
                wi = l * 4 + ti
                bias = bfc[:, wi:wi + 1]
                biasp1 = bfcp1[:, wi:wi + 1]
                for g0 in range(0, n_tiles, OBATCH):
                    nb = min(OBATCH, n_tiles - g0)
                    span = nb * TILE_N
                    itile = p_inp.tile([11, OBATCH * TILE_N], BF16, tag="inp",
                                       name="inp")
                    nc.sync.dma_start(
                        itile[:, 0:span],
                        inp_ap[t][:, g0 * TILE_N:g0 * TILE_N + span])
                    osb = p_out.tile([128, OBATCH * TILE_N], BF16, tag="osb",
                                     name="osb")
                    col0 = tbase + l * PPC[t] + g0 * TILE_N
                    osb_key = object()
                    osb_pending[osb_key] = [osb, col0, nb * (TILE_N // SUB), span]

                    for gg in range(nb):
                        z = ps_z.tile([128, TILE_N], F32, tag="zps", name="zps")
                        for j in range(TILE_N // SUB):
                            off = gg * TILE_N + j * SUB
                            nc.tensor.matmul(
                                z[:, j * SUB:(j + 1) * SUB],
                                lhsT=wpack[:, wi * 128:(wi + 1) * 128],
                                rhs=itile[:, off:off + SUB],
                                start=True, stop=True)
                        # shifted ELU: T1 = ELU(z+b)+1 = min(exp(z+b), max(z+b+1, 1))
                        e1 = p_e1.tile([128, TILE_N], BF16, tag="e1", name="e1")
                        nc.scalar.activation(e1[:], z[:], AF.Exp, bias=bias)
                        t1 = p_t1.tile([128, TILE_N], BF16, tag="t1", name="t1")
                        if (gl_i[0] * 7) % 10 < R1_DVE * 10:
                            r1 = p_r1.tile([128, TILE_N], BF16, tag="r1",
                                           name="r1")
                            nc.vector.tensor_scalar(r1[:], z[:], biasp1, 1.0,
                                                    OP.add, OP.max)
                            ceng = (nc.gpsimd if (gl_i[0] * 3) % 10 < POOL_C * 10
                                    else nc.vector)
                            ceng.tensor_tensor(t1[:], e1[:], r1[:], OP.min)
                        else:
                            r1 = p_r1.tile([128, TILE_N], BF16, tag="r1",
                                           name="r1")
                            nc.scalar.activation(r1[:], z[:], AF.Relu, bias=bias)
                            ceng = (nc.gpsimd if (gl_i[0] * 3) % 10 < POOL_C * 10
                                    else nc.vector)
                            ceng.scalar_tensor_tensor(t1[:], r1[:], 1.0,
                                                      e1[:], OP.add, OP.min)
                        gl_i[0] += 1

                        def stage2(pq_sb, jj, is_pair, osb=osb,
                                   off0=gg * TILE_N, osb_key=osb_key):
                            n2 = 2 if is_pair else 1
                            w = n2 * SUB
                            y = ps_y.tile([128, 2 * SUB], F32, tag="yps",
                                          name="yps")
                            for q in range(n2):
                                jq = jj + q
                                nc.tensor.matmul(
                                    y[:, q * SUB:(q + 1) * SUB],
                                    lhsT=w2c[32 * jq:32 * jq + 2, :],
                                    rhs=pq_sb[32 * jq:32 * jq + 2, :],
                                    start=True, stop=True,
                                    tile_position=(32 * jq, 0))
                            e2 = p_e2.tile([128, 2 * SUB], BF16, tag="e2",
                                           name="e2")
                            nc.scalar.activation(e2[:, 0:w], y[:, 0:w], AF.Exp,
                                                 bias=b2adj[:, 0:1])
                            r2 = p_r2.tile([128, 2 * SUB], BF16, tag="r2",
                                           name="r2")
                            if (pair_i[0] * 7) % 10 < R2_DVE * 10:
                                nc.vector.tensor_scalar(
                                    r2[:, 0:w], y[:, 0:w], b2adjp1[:, 0:1], 1.0,
                                    OP.add, OP.max)
                                ceng = (nc.gpsimd
                                        if (pair_i[0] * 3) % 10 < POOL_C * 10
                                        else nc.vector)
                                ceng.tensor_tensor(
                                    osb[:, off0:off0 + w], e2[:, 0:w],
                                    r2[:, 0:w], OP.min)
                            else:
                                nc.scalar.activation(r2[:, 0:w], y[:, 0:w],
                                                     AF.Relu, bias=b2adj[:, 0:1])
                                ceng = (nc.gpsimd
                                        if (pair_i[0] * 3) % 10 < POOL_C * 10
                                        else nc.vector)
                                ceng.scalar_tensor_tensor(
                                    osb[:, off0:off0 + w], r2[:, 0:w], 1.0,
                                    e2[:, 0:w], OP.add, OP.min)
                            pair_i[0] += 1
                            for _ in range(n2):
                                osb_done_one(osb_key)

                        for j in range(TILE_N // SUB):
                            add_to_pack(t1, j * SUB, stage2)

        flush_pack()
        assert not osb_pending


def _prep_inputs(x_SB, c_SB, x_PQ, c_PQ, x_PV, c_PV, x_NB, c_NB,
                 W_fc, b_fc, W2, b2):
    bf = ml_dtypes.bfloat16
    xs = {"SB": x_SB, "PQ": x_PQ, "PV": x_PV, "NB": x_NB}
    cs = {"SB": c_SB, "PQ": c_PQ, "PV": c_PV, "NB": c_NB}
    # per-core padded input: core i's VPC valid nodes at the front of its
    # PPC-wide slab, zero padded.
    inp = {}
    for t in NODE_TYPES:
        a = np.zeros((N_CORES, 11, PPC[t]), dtype=bf)
        xT = xs[t].T.astype(bf)
        cT = cs[t].T.astype(bf)
        v = VPC[t]
        for i in range(N_CORES):
            a[i, :4, :v] = xT[:, i * v:(i + 1) * v]
            a[i, 4:11, :v] = cT[:, i * v:(i + 1) * v]
        inp[t] = a
    wpack = np.zeros((11, NUM_LAYERS * 4 * 128), dtype=bf)
    bfc = np.zeros((128, NUM_LAYERS * 4), dtype=np.float32)
    for l in range(NUM_LAYERS):
        for ti in range(4):
            wpack[:, (l * 4 + ti) * 128:(l * 4 + ti + 1) * 128] = \
                W_fc[l, ti].astype(bf)
            bfc[:, l * 4 + ti] = b_fc[l, ti].astype(np.float32)
    mask = np.zeros((128, 2), dtype=bf)
    mask[:64, 0] = 1.0
    mask[64:, 1] = 1.0
    w2c = np.zeros((98, 128), dtype=np.float32)
    for j in range(4):
        w2c[32 * j:32 * j + 2, :] = W2.astype(np.float32)
    bfcp1 = bfc + 1.0
    w2f = W2.astype(np.float32)
    b2adj = (b2.astype(np.float32) - 64.0 * (w2f[0] + w2f[1])).reshape(128, 1)
    b2adjp1 = b2adj + 1.0

    in_maps = []
    for i in range(N_CORES):
        m = {f"inp_{t}": inp[t][i] for t in NODE_TYPES}
        m.update(wpack=wpack, bfc=bfc, bfcp1=bfcp1, mask=mask, w2c=w2c,
                 b2adj=b2adj, b2adjp1=b2adjp1)
        in_maps.append(m)
    return in_maps


def kernel(**inputs):
    if "nc" not in _CACHE:
        _CACHE["nc"] = _build_nc()
    nc = _CACHE["nc"]
    in_maps = _prep_inputs(**inputs)
    trace = bool(int(os.environ.get("K_TRACE", "0")))
    res = run_bass_kernel_spmd(nc, in_maps, core_ids=list(range(N_CORES)),
                               trace=trace)
    _CACHE["last_result"] = res
    outs = res.results if hasattr(res, "results") else res

    full = np.empty((NUM_LAYERS * sum(SIZES.values()), 128), dtype=np.float32)
    row = 0
    type_row0 = {}
    for t in NODE_TYPES:
        type_row0[t] = row
        row += NUM_LAYERS * SIZES[t]
    for i in range(N_CORES):
        o = np.asarray(outs[i]["out"])           # [128, OUT_COLS] bf16
        oT = o.T.astype(np.float32) - 1.0        # out stored as ELU+1
        base = 0
        for t in NODE_TYPES:
            for l in range(NUM_LAYERS):
                src = base + l * PPC[t]
                dst = type_row0[t] + l * SIZES[t] + i * VPC[t]
                full[dst:dst + VPC[t]] = oT[src:src + VPC[t]]
            base += NUM_LAYERS * PPC[t]
    return full



# revision 13
# speedup vs baseline: 1.5440x; 1.5440x over previous
"""Trainium2 Bass kernel for nn_ACOPFEnforcer (per-node-type MLP, no message passing).

Math per node type t (sizes SB=4000, PQ=200000, PV=80000, NB=116000):
    inp = concat(x_t, c_t)                      # [N, 11]
    z_l = inp @ W[l,t] + b[l,t]                 # l = 0..2, [N, 128]
    h_l = ELU(z_l)
    P_l = h_l[:, :64].sum(-1); Q_l = h_l[:, 64:].sum(-1)
    out[l*N+n] = ELU(P_l[n]*W2[0] + Q_l[n]*W2[1] + b2)   # [128]
Output = concat over types of the [3*N_t, 128] blocks.

Strategy: pure data parallelism over nodes across 8 NeuronCores.  On-chip
everything runs channels-on-partitions, nodes-on-free-dim.  ELU is computed
exactly via the shifted identity  ELU(z)+1 = max(z+b+1, min(exp(z+b), 1)):
one ScalarE Exp plus ONE fused custom-DVE op (registered at import time
via the documented dve_ops extension point) that computes
max(in0 + s0, min(in1, 1)) in a single Vector pass.  A tunable fraction
of tiles instead runs ScalarE Relu + a Pool-engine scalar_tensor_tensor
(SBUF-only, since GPSIMD cannot read PSUM) to balance the three engines.

Key fusion: stage-2 y[c,n] = w0[c]*P[n] + w1[c]*Q[n] = sum_i M[i,c]*t1[i,n]
with M[i,c] = W2[0,c] (i<64) / W2[1,c] (i>=64), because P/Q are plain
column-half sums of t1 = ELU+1 (the +1 offset folds into b2adj as
b2 - 64*(w0+w1)).  So the P/Q mask-matmul, PSUM quadrant packing, drain
copies, and separate stage-2 matmul of the old design collapse into a
single K=128 matmul per 512 nodes.

Output is written bf16 in [channel, node] layout (values stored as ELU+1)
and transposed/cast/-1 on the host.
"""

import os
import numpy as np
import ml_dtypes

import concourse.bass as bass
import concourse.tile as tile
from concourse import bacc, mybir
from concourse.bass_utils import run_bass_kernel_spmd

BF16 = mybir.dt.bfloat16
F32 = mybir.dt.float32
AF = mybir.ActivationFunctionType
OP = mybir.AluOpType

NODE_TYPES = ["SB", "PQ", "PV", "NB"]
SIZES = {"SB": 4000, "PQ": 200000, "PV": 80000, "NB": 116000}
NUM_LAYERS = 3
N_CORES = 8
TILE_N = 1024          # nodes per unit (z/y psum tile width, 2 banks)
SUB = 512              # nodes per matmul (one PSUM bank of fp32)
# padded per-type sizes: multiple of 8*TILE_N so each core gets whole tiles
PAD = {t: -(-SIZES[t] // (N_CORES * TILE_N)) * (N_CORES * TILE_N) for t in NODE_TYPES}
PPC = {t: PAD[t] // N_CORES for t in NODE_TYPES}        # padded nodes/core
VPC = {t: SIZES[t] // N_CORES for t in NODE_TYPES}      # valid nodes/core
OUT_COLS = sum(NUM_LAYERS * PPC[t] for t in NODE_TYPES)  # per-core out cols

# fraction of units routed to the C-path (ScalarE Relu + stt combine) instead
# of the F-path (fused custom DVE op on Vector)
C_FRAC = float(os.environ.get("K_C", "0.0"))

_CACHE = {}


def _register_elu_op():
    """Register the fused ELU combine as a custom DVE op:
    out = max(in0 + s0, min(in1, 1))   [in0: z psum f32, in1: exp(z+b) bf16]
    This is ELU(z+b)+1 when s0 = b+1 and in1 = exp(z+b)."""
    if "elu_op" in _CACHE:
        return _CACHE["elu_op"]
    import concourse.dve_ops as dve_ops
    from concourse.dve_spec import Spec, Src0, Src1, C0, One, maxx, minn, lower
    from concourse.dve_uop import DveOpSpec

    name = "ELU_SHIFT_COMBINE_ANT"
    body = maxx(Src0 + C0, minn(Src1, One))
    spec = Spec(
        body=body,
        reference=lambda in0, in1, c0, c1, c2: np.maximum(
            np.asarray(in0, np.float32) + c0,
            np.minimum(np.asarray(in1, np.float32), 1.0)),
    )
    if name not in dve_ops._SUB_OPCODE_FOR_NAME:
        row = max(dve_ops._SUB_OPCODE_FOR_NAME.values()) + 1
        assert row < 0x20
        dve_ops._SUB_OPCODE_FOR_NAME[name] = row
    # pin the sha the same way DveOp.compile derives it
    shas = {}
    for ver in ("v3", "v4"):
        s = DveOpSpec(name=name, opcode=dve_ops._SUB_OPCODE_FOR_NAME[name],
                      uops=lower(spec, ver=ver), rd1_en=True)
        shas[ver] = s.sha(ver)
    op = dve_ops.DveOp(name, spec, subdim=False, uops_sha=shas)
    if not any(o.name == name for o in dve_ops.OPS):
        dve_ops.OPS.append(op)
    dve_ops.CUSTOM_DVE_SPECS[name] = spec
    _CACHE["elu_op"] = op
    return op


def _build_nc():
    _register_elu_op()
    nc = bacc.Bacc("TRN2", target_bir_lowering=False, debug=False,
                   enable_asserts=False, num_devices=N_CORES)

    inp_ap = {t: nc.dram_tensor(f"inp_{t}", [11, PPC[t]], BF16,
                                kind="ExternalInput").ap() for t in NODE_TYPES}
    wpack_ap = nc.dram_tensor("wpack", [11, NUM_LAYERS * 4 * 128], BF16,
                              kind="ExternalInput").ap()
    bfc_ap = nc.dram_tensor("bfc", [128, NUM_LAYERS * 4], F32,
                            kind="ExternalInput").ap()
    bfcp1_ap = nc.dram_tensor("bfcp1", [128, NUM_LAYERS * 4], F32,
                              kind="ExternalInput").ap()
    m2_ap = nc.dram_tensor("m2", [128, 128], BF16, kind="ExternalInput").ap()
    b2adj_ap = nc.dram_tensor("b2adj", [128, 1], F32, kind="ExternalInput").ap()
    b2adjp1_ap = nc.dram_tensor("b2adjp1", [128, 1], F32,
                                kind="ExternalInput").ap()
    out_ap = nc.dram_tensor("out", [128, OUT_COLS], BF16,
                            kind="ExternalOutput").ap()

    with tile.TileContext(nc) as tc:
        _emit(tc, inp_ap, wpack_ap, bfc_ap, bfcp1_ap, m2_ap, b2adj_ap,
              b2adjp1_ap, out_ap)
    nc.compile()
    return nc


def _emit(tc, inp_ap, wpack_ap, bfc_ap, bfcp1_ap, m2_ap, b2adj_ap,
          b2adjp1_ap, out_ap):
    nc = tc.nc
    from contextlib import ExitStack
    ctx = ExitStack()
    with ctx:
        elu_op = _CACHE["elu_op"]
        consts = ctx.enter_context(tc.tile_pool(name="consts", bufs=1))
        p_inp = ctx.enter_context(tc.tile_pool(name="inp", bufs=3))
        p_e1 = ctx.enter_context(tc.tile_pool(name="e1", bufs=3))
        p_r1 = ctx.enter_context(tc.tile_pool(name="r1", bufs=3))
        p_t1 = ctx.enter_context(tc.tile_pool(name="t1", bufs=3))
        p_e2 = ctx.enter_context(tc.tile_pool(name="e2", bufs=3))
        p_r2 = ctx.enter_context(tc.tile_pool(name="r2", bufs=3))
        p_out = ctx.enter_context(tc.tile_pool(name="osb", bufs=3))
        ps_z = ctx.enter_context(tc.tile_pool(name="zps", bufs=2, space="PSUM"))
        ps_y = ctx.enter_context(tc.tile_pool(name="yps", bufs=2, space="PSUM"))

        # constants
        wpack = consts.tile([11, NUM_LAYERS * 4 * 128], BF16, tag="wpack",
                            name="wpack")
        nc.sync.dma_start(wpack[:], wpack_ap[:])
        bfc = consts.tile([128, NUM_LAYERS * 4], F32, tag="bfc", name="bfc")
        nc.sync.dma_start(bfc[:], bfc_ap[:])
        bfcp1 = consts.tile([128, NUM_LAYERS * 4], F32, tag="bfcp1",
                            name="bfcp1")
        nc.sync.dma_start(bfcp1[:], bfcp1_ap[:])
        m2 = consts.tile([128, 128], BF16, tag="m2", name="m2")
        nc.sync.dma_start(m2[:], m2_ap[:])
        b2adj = consts.tile([128, 1], F32, tag="b2adj", name="b2adj")
        nc.sync.dma_start(b2adj[:], b2adj_ap[:])
        b2adjp1 = consts.tile([128, 1], F32, tag="b2adjp1", name="b2adjp1")
        nc.sync.dma_start(b2adjp1[:], b2adjp1_ap[:])

        OBATCH = 4   # units batched into one input/output DMA

        path_i = [0]   # C-path routing counter

        def use_cpath():
            k = path_i[0]
            path_i[0] += 1
            return (k * 7) % 10 < C_FRAC * 10

        def elu_stage(src_ps, dst, biasap, biasp1ap, p_e, p_r, etag, rtag):
            """dst = ELU(src+bias)+1 from PSUM src, via F- or C-path."""
            e = p_e.tile([128, TILE_N], BF16, tag=etag, name=etag)
            if use_cpath():
                nc.scalar.activation(e[:], src_ps[:], AF.Exp, bias=biasap)
                r = p_r.tile([128, TILE_N], BF16, tag=rtag, name=rtag)
                nc.scalar.activation(r[:], src_ps[:], AF.Relu, bias=biasap)
                # out = min(r+1, e)
                nc.vector.scalar_tensor_tensor(dst, r[:], 1.0, e[:],
                                               OP.add, OP.min)
            else:
                nc.scalar.activation(e[:], src_ps[:], AF.Exp, bias=biasap)
                # out = max(src + (bias+1), min(e, 1))  (one fused DVE op)
                nc.vector._custom_dve(elu_op, out=dst, in0=src_ps[:],
                                      in1=e[:], s0=biasp1ap)

        for ti, t in enumerate(NODE_TYPES):
            n_tiles = PPC[t] // TILE_N
            tbase = sum(NUM_LAYERS * PPC[u] for u in NODE_TYPES[:ti])
            for l in range(NUM_LAYERS):
                wi = l * 4 + ti
                bias = bfc[:, wi:wi + 1]
                biasp1 = bfcp1[:, wi:wi + 1]
                for g0 in range(0, n_tiles, OBATCH):
                    nb = min(OBATCH, n_tiles - g0)
                    span = nb * TILE_N
                    itile = p_inp.tile([11, OBATCH * TILE_N], BF16, tag="inp",
                                       name="inp")
                    nc.sync.dma_start(
                        itile[:, 0:span],
                        inp_ap[t][:, g0 * TILE_N:g0 * TILE_N + span])
                    osb = p_out.tile([128, OBATCH * TILE_N], BF16, tag="osb",
                                     name="osb")
                    col0 = tbase + l * PPC[t] + g0 * TILE_N

                    for gg in range(nb):
                        off = gg * TILE_N
                        z = ps_z.tile([128, TILE_N], F32, tag="zps",
                                      name="zps")
                        for j in range(TILE_N // SUB):
                            nc.tensor.matmul(
                                z[:, j * SUB:(j + 1) * SUB],
                                lhsT=wpack[:, wi * 128:(wi + 1) * 128],
                                rhs=itile[:, off + j * SUB:
                                          off + (j + 1) * SUB],
                                start=True, stop=True)
                        t1 = p_t1.tile([128, TILE_N], BF16, tag="t1",
                                       name="t1")
                        elu_stage(z, t1[:], bias, biasp1, p_e1, p_r1,
                                  "e1", "r1")
                        y = ps_y.tile([128, TILE_N], F32, tag="yps",
                                      name="yps")
                        for j in range(TILE_N // SUB):
                            nc.tensor.matmul(
                                y[:, j * SUB:(j + 1) * SUB],
                                lhsT=m2[:, :],
                                rhs=t1[:, j * SUB:(j + 1) * SUB],
                                start=True, stop=True)
                        elu_stage(y, osb[:, off:off + TILE_N],
                                  b2adj[:, 0:1], b2adjp1[:, 0:1], p_e2, p_r2,
                                  "e2", "r2")
                    nc.sync.dma_start(out_ap[:, col0:col0 + span],
                                      osb[:, 0:span])


def _prep_inputs(x_SB, c_SB, x_PQ, c_PQ, x_PV, c_PV, x_NB, c_NB,
                 W_fc, b_fc, W2, b2):
    bf = ml_dtypes.bfloat16
    xs = {"SB": x_SB, "PQ": x_PQ, "PV": x_PV, "NB": x_NB}
    cs = {"SB": c_SB, "PQ": c_PQ, "PV": c_PV, "NB": c_NB}
    # per-core padded input: core i's VPC valid nodes at the front of its
    # PPC-wide slab, zero padded.
    inp = {}
    for t in NODE_TYPES:
        a = np.zeros((N_CORES, 11, PPC[t]), dtype=bf)
        xT = xs[t].T.astype(bf)
        cT = cs[t].T.astype(bf)
        v = VPC[t]
        for i in range(N_CORES):
            a[i, :4, :v] = xT[:, i * v:(i + 1) * v]
            a[i, 4:11, :v] = cT[:, i * v:(i + 1) * v]
        inp[t] = a
    wpack = np.zeros((11, NUM_LAYERS * 4 * 128), dtype=bf)
    bfc = np.zeros((128, NUM_LAYERS * 4), dtype=np.float32)
    for l in range(NUM_LAYERS):
        for ti in range(4):
            wpack[:, (l * 4 + ti) * 128:(l * 4 + ti + 1) * 128] = \
                W_fc[l, ti].astype(bf)
            bfc[:, l * 4 + ti] = b_fc[l, ti].astype(np.float32)
    bfcp1 = bfc + 1.0
    w2f = W2.astype(np.float32)
    # fused stage-2 weight: y[c,n] = sum_i m2[i,c] * t1[i,n]
    m2 = np.zeros((128, 128), dtype=bf)
    m2[:64, :] = w2f[0][None, :].astype(bf)
    m2[64:, :] = w2f[1][None, :].astype(bf)
    # the matmul of t1 = h+1 adds exactly sum_i m2[i,c] (the ROUNDED
    # weights); correct with that same rounded sum, not the exact one
    b2adj = (b2.astype(np.float32)
             - m2.astype(np.float32).sum(axis=0)).reshape(128, 1)
    b2adjp1 = b2adj + 1.0

    in_maps = []
    for i in range(N_CORES):
        m = {f"inp_{t}": inp[t][i] for t in NODE_TYPES}
        m.update(wpack=wpack, bfc=bfc, bfcp1=bfcp1, m2=m2,
                 b2adj=b2adj, b2adjp1=b2adjp1)
        in_maps.append(m)
    return in_maps


def kernel(**inputs):
    if "nc" not in _CACHE:
        _CACHE["nc"] = _build_nc()
    nc = _CACHE["nc"]
    in_maps = _prep_inputs(**inputs)
    trace = bool(int(os.environ.get("K_TRACE", "0")))
    res = run_bass_kernel_spmd(nc, in_maps, core_ids=list(range(N_CORES)),
                               trace=trace)
    _CACHE["last_result"] = res
    outs = res.results if hasattr(res, "results") else res

    full = np.empty((NUM_LAYERS * sum(SIZES.values()), 128), dtype=np.float32)
    row = 0
    type_row0 = {}
    for t in NODE_TYPES:
        type_row0[t] = row
        row += NUM_LAYERS * SIZES[t]
    for i in range(N_CORES):
        o = np.asarray(outs[i]["out"])           # [128, OUT_COLS] bf16
        oT = o.T.astype(np.float32) - 1.0        # out stored as ELU+1
        base = 0
        for t in NODE_TYPES:
            for l in range(NUM_LAYERS):
                src = base + l * PPC[t]
                dst = type_row0[t] + l * SIZES[t] + i * VPC[t]
                full[dst:dst + VPC[t]] = oT[src:src + VPC[t]]
            base += NUM_LAYERS * PPC[t]
    return full
